# revision 16
# baseline (speedup 1.0000x reference)
"""Trainium2 Bass kernel for nn_DN1 (vq_codebook).

Reference semantics (see analysis):
    inpt = concat(x, y_response, z)                         # (9216,)
    dot  = neurons @ inpt                                   # (5120,)
    response = relu(dot - 0.5); active = dot > 0.5
    The two masked updates (z rows then y rows) are independent because the
    masks are disjoint and each weighted row-sum only reads rows of its own
    group (unmodified at that point).  So with s_g = sum over active rows of
    group g of ((ages-1)/ages) * neurons[r]:
        active row r:  new_r = normalize(s_{g_r} + inpt/ages_r)
                             = (ages_r*s_{g_r} + inpt) / (||ages_r*s_{g_r} + inpt|| + ages_r*1e-10)
        ages_out = ages + active
    z_response = response[4096:5120]

Distribution: rows sharded 640/core over 8 cores; one AllReduce of the
partial s vectors [2, 9216] per step.  norm^2 computed algebraically:
    ||a*s+i||^2 = a^2*S2_sel + 2a*SI_sel + I2
from the Gram matrix of [s_y; s_z; inpt] (accumulated on the PE).
"""

import sys

sys.path.insert(0, "/opt/trn_rl_repo")
sys.path.insert(0, "/opt/pypackages")

from contextlib import ExitStack

import numpy as np

import concourse.bass as bass
import concourse.tile as tile
from concourse import bacc, mybir
from concourse.bass_utils import run_bass_kernel_spmd

# problem constants (hardcoded per harness contract)
X_SIZE = 4096
NUM_NEURONS = 4096
Z_SIZE = 1024
ROWS = NUM_NEURONS + Z_SIZE          # 5120
COLS = X_SIZE + NUM_NEURONS + Z_SIZE  # 9216
THRESH = 0.5

N_CORES = 8
RPC = ROWS // N_CORES                 # 640 rows per core
P = 128
NT = RPC // P                         # 5 tiles per core
CH = 512                              # column chunk
NC_CH = COLS // CH                    # 18 chunks
THIRD = 1536                          # broadcast buffer segment
NTH = COLS // THIRD                   # 6 segments
CPT = THIRD // CH                     # 3 chunks per segment

F32 = mybir.dt.float32
F32R = mybir.dt.float32r
Alu = mybir.AluOpType
Act = mybir.ActivationFunctionType


def build_kernel():
    nc = bacc.Bacc("TRN2", target_bir_lowering=False, debug=False,
                   num_devices=N_CORES)

    neur = nc.dram_tensor("neur", [RPC, COLS], F32, kind="ExternalInput")
    ages = nc.dram_tensor("ages", [RPC], F32, kind="ExternalInput")
    grp = nc.dram_tensor("grp", [RPC], F32, kind="ExternalInput")
    inpt = nc.dram_tensor("inpt", [COLS], F32, kind="ExternalInput")

    neur_out = nc.dram_tensor("neur_out", [RPC, COLS], F32, kind="ExternalOutput")
    ages_out = nc.dram_tensor("ages_out", [RPC], F32, kind="ExternalOutput")
    resp_out = nc.dram_tensor("resp_out", [RPC], F32, kind="ExternalOutput")

    s_wire_in = nc.dram_tensor("s_wire_in", [2, COLS], F32)
    s_wire_out = nc.dram_tensor("s_wire_out", [2, COLS], F32, addr_space="Shared")
    qt_bounce = nc.dram_tensor("qt_bounce", [NT, P, 3], F32)

    with tile.TileContext(nc) as tc, ExitStack() as ctx:
        resid_p = ctx.enter_context(tc.tile_pool(name="resid", bufs=1))
        bc_p = ctx.enter_context(tc.tile_pool(name="bc", bufs=1))
        small_p = ctx.enter_context(tc.tile_pool(name="small", bufs=1))
        r32_p = ctx.enter_context(tc.tile_pool(name="r32", bufs=1))
        psum_p = ctx.enter_context(tc.tile_pool(name="psum", bufs=4, space="PSUM"))
        psum_tt = ctx.enter_context(tc.tile_pool(name="psumtt", bufs=2, space="PSUM"))
        psum_g = ctx.enter_context(tc.tile_pool(name="psumg", bufs=1, space="PSUM"))

        nb = resid_p.tile([P, NT, COLS], F32)          # 184,320 B/part
        bc = bc_p.tile([P, THIRD], F32)                # 6,144
        sm = small_p.tile([P, 1664], F32)              # 6,656
        coef2_r = r32_p.tile([P, 2 * NT], F32R, tag="coef2r")   # 4,096
        chunk_r0 = r32_p.tile([P, CH], F32R, tag="chr0")        # 4,096
        chunk_r1 = r32_p.tile([P, CH], F32R, tag="chr1")        # 4,096
        chunk_r = [chunk_r0, chunk_r1]
        lhsT3_r = r32_p.tile([3, NT * P], F32R, tag="lhsT3r")   # 4,096 (pad)
        s3_r = r32_p.tile([3, CH], F32R, tag="s3r")             # 4,096 (pad)

        # ---- small-tensor column map (all slices of sm [P, 2048]) ----
        AGES, GRP_, GY, DOT, RESP, M, ONEM = 0, 5, 10, 15, 20, 25, 30
        COEF2 = 35     # [P, 2] per tile            35..45
        Q3A = 45       # [P, 3] per tile            45..60   (a*gy, a*g, 1)
        Q3B = 60       # [P, 3] per tile            60..75
        SC = 75        # per-row scratch            75..95
        GBC = 95       # bcast S2y,S2z,SIy,SIz,I2   95..100
        PART5 = 100    # per-partition partials     100..105
        ONES = 105     # ones column                105..106
        ACCS = 128     # ACT dot partials [P, 90]   128..218
        SPM = 220      # s p-minor [P, 2, 72]       220..364  (phase 3)
        SQSC = 364     # square scratch [P, 72]     364..436
        INPM = 436     # inpt p-minor [P, 72]       436..508
        LHS3F = 512    # [3, 128] staging slot      512..640  (parts 0..2)
        # Regions R1/R2 are time-shared: phase 2 wire staging [2, CH], then
        # phase 3/4 s3 staging [3, CH].
        R1 = 640       # 640..1152
        R2 = 1152      # 1152..1664

        def col(c0, t=0, w=1):
            return sm[:, c0 + t * w: c0 + (t + 1) * w]

        # ================= phase 0: loads =================
        nc.sync.dma_start(sm[:, AGES:AGES + NT],
                          ages.ap().rearrange("(t p) -> p t", p=P))
        nc.sync.dma_start(sm[:, GRP_:GRP_ + NT],
                          grp.ap().rearrange("(t p) -> p t", p=P))
        nc.sync.dma_start(sm[:, INPM:INPM + 72],
                          inpt.ap().rearrange("(p f) -> p f", f=72))
        for t in range(NT):
            nc.sync.dma_start(nb[:, t, :], neur[t * P:(t + 1) * P, :])

        # gy = 1 - g (per tile)
        for t in range(NT):
            nc.vector.tensor_scalar(col(GY, t), col(GRP_, t), -1.0, 1.0,
                                    Alu.mult, Alu.add)

        # ================= phase 1: response =================
        inpt_ap = inpt.ap()

        def inpt_slice(lo, n):
            sl = inpt_ap[lo:lo + n]
            return bass.AP(sl.tensor, sl.offset, [[1, 1], [1, n]])

        def inpt_bcast(lo, n):
            sl = inpt_ap[lo:lo + n]
            return bass.AP(sl.tensor, sl.offset, [[0, P], [1, n]])

        for h in range(NTH):
            nc.sync.dma_start(bc[:, :], inpt_bcast(h * THIRD, THIRD))
            for t in range(NT):
                for c in range(CPT):
                    cc = h * CPT + c
                    prod = psum_tt.tile([P, CH], F32, tag="prod")
                    nc.vector.tensor_tensor(
                        prod[:], nb[:, t, cc * CH:(cc + 1) * CH],
                        bc[:, c * CH:(c + 1) * CH], Alu.mult)
                    nc.scalar.activation(
                        prod[:], prod[:], Act.Copy,
                        accum_out=sm[:, ACCS + t * NC_CH + cc:
                                     ACCS + t * NC_CH + cc + 1])

        for t in range(NT):
            # dot, response, masks, coefs
            nc.vector.tensor_reduce(
                col(DOT, t), sm[:, ACCS + t * NC_CH:ACCS + (t + 1) * NC_CH],
                mybir.AxisListType.X, Alu.add)
            nc.vector.tensor_scalar(col(RESP, t), col(DOT, t), -THRESH, 0.0,
                                    Alu.add, Alu.max)
            nc.vector.tensor_scalar(col(M, t), col(DOT, t), THRESH, None,
                                    Alu.is_gt)
            nc.vector.tensor_scalar(col(ONEM, t), col(M, t), -1.0, 1.0,
                                    Alu.mult, Alu.add)
            # coef = m * (1 - 1/a)
            ra = col(SC, 0, 1)
            nc.vector.reciprocal(ra, col(AGES, t))
            cf = col(SC, 1, 1)
            nc.vector.tensor_scalar(cf, ra, -1.0, 1.0, Alu.mult, Alu.add)
            nc.vector.tensor_scalar(cf, cf, col(M, t), None, Alu.mult)
            c2 = sm[:, COEF2 + 2 * t:COEF2 + 2 * t + 2]
            nc.vector.tensor_scalar(c2[:, 0:1], cf, col(GY, t), None, Alu.mult)
            nc.vector.tensor_scalar(c2[:, 1:2], cf, col(GRP_, t), None, Alu.mult)
            nc.vector.tensor_copy(coef2_r[:, 2 * t:2 * t + 2], c2)
            # q3A = (a*gy, a*g, 1)
            q3a = sm[:, Q3A + 3 * t:Q3A + 3 * t + 3]
            nc.vector.tensor_scalar(q3a[:, 0:1], col(GY, t), col(AGES, t),
                                    None, Alu.mult)
            nc.vector.tensor_scalar(q3a[:, 1:2], col(GRP_, t), col(AGES, t),
                                    None, Alu.mult)
            nc.vector.memset(q3a[:, 2:3], 1.0)
            # outputs: ages_out, resp_out
            ao = col(SC, 2, 1)
            nc.vector.tensor_scalar(ao, col(AGES, t), col(M, t), None, Alu.add)
            nc.sync.dma_start(
                ages_out.ap().rearrange("(t p) -> p t", p=P)[:, t:t + 1], ao)
            nc.sync.dma_start(
                resp_out.ap().rearrange("(t p) -> p t", p=P)[:, t:t + 1],
                col(RESP, t))

        # ================= phase 2: s matvec =================
        for cc in range(NC_CH):
            ps = psum_p.tile([2, CH], F32, tag="mm")
            for t in range(NT):
                cr = chunk_r[(cc * NT + t) % 2]
                nc.vector.tensor_copy(cr[:], nb[:, t, cc * CH:(cc + 1) * CH])
                nc.tensor.matmul(ps[:], coef2_r[:, 2 * t:2 * t + 2], cr[:],
                                 start=(t == 0), stop=(t == NT - 1))
            r = R1 if cc % 2 == 0 else R2
            st = sm[0:2, r:r + CH]
            nc.scalar.activation(st, ps[:], Act.Copy)
            nc.sync.dma_start(s_wire_in[:, cc * CH:(cc + 1) * CH], st)

        # ================= collective =================
        nc.gpsimd.collective_compute(
            "AllReduce", Alu.add,
            replica_groups=[list(range(N_CORES))],
            ins=[s_wire_in[:, :]], outs=[s_wire_out[:, :]])

        # ================= phase 3: norm scalars + per-row scalars =========
        # side-load global s in a p-minor layout (order-agnostic for sums)
        nc.sync.dma_start(sm[:, SPM:SPM + 144],
                          s_wire_out.ap().rearrange("g (p f) -> p g f", f=72))
        sq = sm[:, SQSC:SQSC + 72]
        for g in range(2):
            sg = sm[:, SPM + g * 72:SPM + (g + 1) * 72]
            nc.vector.tensor_tensor(sq, sg, sg, Alu.mult)
            nc.scalar.activation(sq, sq, Act.Copy,
                                 accum_out=sm[:, PART5 + g:PART5 + g + 1])
            nc.vector.tensor_tensor(sq, sg, sm[:, INPM:INPM + 72], Alu.mult)
            nc.scalar.activation(sq, sq, Act.Copy,
                                 accum_out=sm[:, PART5 + 2 + g:PART5 + 3 + g])
        nc.vector.tensor_tensor(sq, sm[:, INPM:INPM + 72],
                                sm[:, INPM:INPM + 72], Alu.mult)
        nc.scalar.activation(sq, sq, Act.Copy,
                             accum_out=sm[:, PART5 + 4:PART5 + 5])
        # cross-partition reduce via ones-matmul (plain fp32: exact, trivial)
        nc.vector.memset(sm[:, ONES:ONES + 1], 1.0)
        pg = psum_g.tile([1, 8], F32, tag="gram")
        nc.tensor.matmul(pg[0:1, 0:5], sm[:, ONES:ONES + 1],
                         sm[:, PART5:PART5 + 5], start=True, stop=True)
        nc.scalar.activation(sm[0:1, GBC:GBC + 5], pg[0:1, 0:5], Act.Copy)
        nc.gpsimd.partition_broadcast(sm[:, GBC:GBC + 5], sm[0:1, GBC:GBC + 5])
        S2Y, S2Z = GBC + 0, GBC + 1
        SIY, SIZ = GBC + 2, GBC + 3
        I2 = GBC + 4

        for t in range(NT):
            s2s = col(SC, 3, 1)   # S2_sel
            sis = col(SC, 4, 1)   # SI_sel
            tmp = col(SC, 5, 1)
            nc.vector.tensor_scalar(s2s, col(GY, t), sm[:, S2Y:S2Y + 1],
                                    None, Alu.mult)
            nc.vector.tensor_scalar(tmp, col(GRP_, t), sm[:, S2Z:S2Z + 1],
                                    None, Alu.mult)
            nc.vector.tensor_add(s2s, s2s, tmp)
            nc.vector.tensor_scalar(sis, col(GY, t), sm[:, SIY:SIY + 1],
                                    None, Alu.mult)
            nc.vector.tensor_scalar(tmp, col(GRP_, t), sm[:, SIZ:SIZ + 1],
                                    None, Alu.mult)
            nc.vector.tensor_add(sis, sis, tmp)
            # norm2 = a*(a*S2 + 2*SI) + I2
            n2 = col(SC, 6, 1)
            nc.vector.tensor_scalar(n2, s2s, col(AGES, t), None, Alu.mult)
            nc.vector.tensor_scalar(tmp, sis, 2.0, None, Alu.mult)
            nc.vector.tensor_add(n2, n2, tmp)
            nc.vector.tensor_scalar(n2, n2, col(AGES, t), None, Alu.mult)
            nc.vector.tensor_scalar(n2, n2, sm[:, I2:I2 + 1], None, Alu.add)
            nrm = col(SC, 7, 1)
            nc.scalar.sqrt(nrm, n2)
            # denom = norm + a*1e-10 ; w = m / denom
            nc.vector.tensor_scalar(tmp, col(AGES, t), 1e-10, None, Alu.mult)
            nc.vector.tensor_add(nrm, nrm, tmp)
            w = col(SC, 8, 1)
            nc.vector.reciprocal(w, nrm)
            nc.vector.tensor_scalar(w, w, col(M, t), None, Alu.mult)
            # q3B = q3A * w  -> bounce-transpose -> lhsT3_f -> round
            q3a = sm[:, Q3A + 3 * t:Q3A + 3 * t + 3]
            q3b = sm[:, Q3B + 3 * t:Q3B + 3 * t + 3]
            nc.vector.tensor_scalar(q3b, q3a, w, None, Alu.mult)
            nc.sync.dma_start(qt_bounce[t, :, :], q3b)
            l3f = sm[0:3, LHS3F:LHS3F + P]
            nc.sync.dma_start(l3f, qt_bounce[t].rearrange("p k -> k p"))
            nc.vector.tensor_copy(lhsT3_r[:, t * P:(t + 1) * P], l3f)

        # ================= phase 4: update + blend + write =================
        for cc in range(NC_CH):
            r = R1 if cc % 2 == 0 else R2
            s3f = sm[0:3, r:r + CH]
            nc.sync.dma_start(s3f[0:2, :], s_wire_out[:, cc * CH:(cc + 1) * CH])
            nc.sync.dma_start(s3f[2:3, :], inpt_slice(cc * CH, CH))
            nc.vector.tensor_copy(s3_r[:], s3f)
            for t in range(NT):
                pn = psum_p.tile([P, CH], F32, tag="mm")
                nc.tensor.matmul(pn[:], lhsT3_r[:, t * P:(t + 1) * P], s3_r[:],
                                 start=True, stop=True)
                chunk = nb[:, t, cc * CH:(cc + 1) * CH]
                nc.scalar.activation(chunk, chunk, Act.Copy,
                                     scale=col(ONEM, t))
                nc.vector.tensor_add(chunk, chunk, pn[:])
                nc.sync.dma_start(
                    neur_out[t * P:(t + 1) * P, cc * CH:(cc + 1) * CH], chunk)

    nc.compile()
    return nc


_NC_CACHE = None


def _get_nc():
    global _NC_CACHE
    if _NC_CACHE is None:
        _NC_CACHE = build_kernel()
    return _NC_CACHE


def make_in_maps(inputs):
    x, y_response, z = inputs["x"], inputs["y_response"], inputs["z"]
    neurons, ages = inputs["neurons"], inputs["ages"]
    inpt = np.ascontiguousarray(
        np.concatenate([np.asarray(x, np.float32),
                        np.asarray(y_response, np.float32),
                        np.asarray(z, np.float32)]))
    neurons = np.asarray(neurons, np.float32)
    ages = np.asarray(ages, np.float32)
    grp_full = (np.arange(ROWS) >= NUM_NEURONS).astype(np.float32)

    in_maps = []
    for k in range(N_CORES):
        sl = slice(k * RPC, (k + 1) * RPC)
        in_maps.append({
            "neur": np.ascontiguousarray(neurons[sl]),
            "ages": np.ascontiguousarray(ages[sl]),
            "grp": np.ascontiguousarray(grp_full[sl]),
            "inpt": inpt,
        })
    return in_maps


def kernel(x, y_response, z, neurons, ages, _trace=False, _trace_kwargs=None):
    nc = _get_nc()
    in_maps = make_in_maps(dict(x=x, y_response=y_response, z=z,
                                neurons=neurons, ages=ages))
    kw = {}
    if _trace:
        kw = dict(trace=True, **(_trace_kwargs or {}))
    res = run_bass_kernel_spmd(nc, in_maps, list(range(N_CORES)), **kw)
    results = res.results

    neurons_out = np.concatenate([results[k]["neur_out"] for k in range(N_CORES)])
    ages_o = np.concatenate([results[k]["ages_out"] for k in range(N_CORES)])
    resp = np.concatenate([results[k]["resp_out"] for k in range(N_CORES)])
    z_response = resp[NUM_NEURONS:NUM_NEURONS + Z_SIZE]
    kernel.last_exec_time_ns = res.exec_time_ns
    return z_response, neurons_out, ages_o


kernel.last_exec_time_ns = None
